# revision 40
# baseline (speedup 1.0000x reference)
"""Diagonal RNN associative scan on 8 TRN2 NeuronCores.

Math (per batch row b, channel p):
    a[p]   = 1 - relu(w[p])
    h[t]   = a[p] * h[t-1] + x[b, t, p],   h[-1] = 0
    out[b, t, p] = h[t]

Strategy:
  - Data-parallel over batch: B=32 rows -> 8 cores x 4 rows. No collectives.
  - Per core, per (batch row, 1024-step scan chunk): DMA in, PE-transpose so
    channels sit on partitions, run the hardware linear-recurrence
    instruction (tensor_tensor_scan) along the free (time) axis in fp32
    carrying h across chunks, PE-transpose back, ACT-copy PSUM->SBUF, DMA out.
  - DMA descriptor-friendly layouts: time rows interleaved per partition so
    descriptors move 2 KiB (in, R=4) / 4 KiB (out, R=8) contiguous instead
    of 512B. The interleave is undone on-chip: in-path transposes write PSUM
    with stride-4 column APs (time-major); out-path transposes read the scan
    output with stride-8 free APs (strided stationary), writing PSUM
    contiguously.
  - Software-pipelined emission: the out-path of chunk k-2 is emitted after
    the in-path of chunk k, so the PE never head-of-line blocks on the scan
    of its own chunk (PE executes its FIFO in order; interleaving keeps it
    busy and lets the clock ramp).
  - in-DMAs issue on the Sync HWDGE ring, out-DMAs alternate between the
    Scalar and Sync rings (a single ring serializes transfers).
"""

import numpy as np

B, L, P = 32, 8192, 128
N_CORES = 8
B_PER = B // N_CORES  # 4 batch rows per core
SC = 1024             # scan-chunk time steps (one DMA + one scan instruction)
G = 512               # in-path interleave group (one PSUM bank of f32)
BLK = 128             # PE transpose tile edge
R_IN = 4              # in-path time-row interleave (2 KiB descriptors).
                      # R_IN=8 f32 needs bank-crossing strided PSUM matmul
                      # writes (NRT_EXEC_UNIT_UNRECOVERABLE on HW); bf16
                      # in-path hits the 4-byte PSUM alignment rule for odd
                      # residues. 2 KiB is the legal in-path maximum.
R_OUT = 8             # out-path time-row interleave (4 KiB descriptors)
PIPE = 3              # out-path emission delay in iterations

_nc_cache = {}


def _build_nc(b_per=B_PER, seq_len=L):
    """Build + compile the per-core Bass program (SPMD; same NEFF on all cores)."""
    import concourse.mybir as mybir
    import concourse.tile as tile
    from concourse import bacc
    from concourse.masks import make_identity

    dt = mybir.dt
    n_ch = seq_len // SC       # scan chunks per batch row
    n_grp = SC // G            # in-path 512-groups per scan chunk
    assert seq_len % SC == 0 and SC % G == 0 and G // BLK == R_IN

    nc = bacc.Bacc("TRN2", target_bir_lowering=False, debug=False)
    x_ext = nc.dram_tensor("x", [b_per, seq_len, P], dt.float32, kind="ExternalInput")
    w_ext = nc.dram_tensor("w", [P, 1], dt.float32, kind="ExternalInput")
    y_ext = nc.dram_tensor("out", [b_per, seq_len, P], dt.float32, kind="ExternalOutput")

    with tile.TileContext(nc) as tc:
        with (
            tc.tile_pool(name="const", bufs=1) as constp,
            tc.tile_pool(name="xin", bufs=8) as inp,
            tc.tile_pool(name="scan", bufs=12) as scanp,
            tc.tile_pool(name="ostg", bufs=6) as outp,
            tc.tile_pool(name="ps_in", bufs=2, space="PSUM") as pinp,
            tc.tile_pool(name="ps_out", bufs=2, space="PSUM") as poutp,
        ):
            ident = constp.tile([P, P], dt.float32, name="ident")
            make_identity(nc, ident[:])

            # w DMA on the gpsimd (SWDGE) ring so the Sync ring's first
            # instruction is iteration 0's x-chunk DMA
            w_col = constp.tile([P, 1], dt.float32, name="w_col")
            nc.gpsimd.dma_start(out=w_col[:], in_=w_ext.ap())
            a_col = constp.tile([P, 1], dt.float32, name="a_col")
            # a = 1 - relu(w)  ==  (max(w, 0) * -1) + 1
            nc.vector.tensor_scalar(
                out=a_col[:], in0=w_col[:], scalar1=0.0, scalar2=None,
                op0=mybir.AluOpType.max,
            )
            nc.vector.tensor_scalar(
                out=a_col[:], in0=a_col[:], scalar1=-1.0, scalar2=1.0,
                op0=mybir.AluOpType.mult, op1=mybir.AluOpType.add,
            )
            # scan's data0 operand: a replicated along the time axis
            a_rep = constp.tile([P, SC], dt.float32, name="a_rep")
            nc.vector.tensor_copy(out=a_rep[:], in_=a_col[:].to_broadcast([P, SC]))

            x_ap = x_ext.ap()
            y_ap = y_ext.ap()
            carry = [None] * b_per
            iters = [(c, b) for c in range(n_ch) for b in range(b_per)]
            state = {}  # k -> s_t tile

            def emit_in(k):
                c, b = iters[k]
                chunk_in = x_ap[b, c * SC:(c + 1) * SC, :]
                xin = inp.tile([P, SC], dt.float32, name="xin")
                if k < 2:
                    # pipeline warm-up: land the chunk in per-group pieces so
                    # the first transposes start after 256 KiB, not 512 KiB
                    for g in range(n_grp):
                        src_g = chunk_in[g * G:(g + 1) * G, :].rearrange(
                            "(tb tl) p -> tb tl p", tl=R_IN
                        )
                        nc.sync.dma_start(out=xin[:, g * G:(g + 1) * G], in_=src_g)
                else:
                    # free = [grp, tl, p]; partition = tb; t = grp*512 + tb*4 + tl
                    src = chunk_in.rearrange(
                        "(grp tb tl) p -> tb grp tl p", grp=n_grp, tl=R_IN
                    )
                    nc.sync.dma_start(out=xin[:], in_=src)

                ps_in = pinp.tile([P, SC], dt.float32, name="ps_in")
                for g in range(n_grp):
                    for tl in range(R_IN):
                        # stride-4 psum columns: time-major within group g
                        nc.tensor.transpose(
                            out=ps_in[:, g * G + tl:(g + 1) * G:R_IN],
                            in_=xin[:, (g * R_IN + tl) * BLK:(g * R_IN + tl + 1) * BLK],
                            identity=ident[:],
                        )

                s_t = scanp.tile([P, SC], dt.float32, name="s_t")
                init = 0.0 if carry[b] is None else carry[b]
                nc.vector.tensor_tensor_scan(
                    out=s_t[:], data0=a_rep[:], data1=ps_in[:],
                    initial=init,
                    op0=mybir.AluOpType.mult, op1=mybir.AluOpType.add,
                )
                carry[b] = s_t[:, SC - 1:SC]
                state[k] = s_t

            def emit_out(k):
                c, b = iters[k]
                s_t = state.pop(k)
                chunk_out = y_ap[b, c * SC:(c + 1) * SC, :]
                if k >= len(iters) - 2:
                    # pipeline drain: process the chunk in two pipelined
                    # halves (R=4, 2 KiB descriptors) so the final DMA isn't
                    # gated on the whole chunk's transposes + copy
                    for h in range(2):
                        ps_h = poutp.tile([P, G], dt.float32, name="ps_out")
                        for tl in range(R_IN):
                            nc.tensor.transpose(
                                out=ps_h[:, tl * BLK:(tl + 1) * BLK],
                                in_=s_t[:, h * G + tl:(h + 1) * G:R_IN],
                                identity=ident[:],
                            )
                        ostg_h = outp.tile([P, G], dt.float32, name="ostg")
                        nc.scalar.activation(
                            out=ostg_h[:], in_=ps_h[:],
                            func=mybir.ActivationFunctionType.Copy,
                        )
                        dst_h = chunk_out[h * G:(h + 1) * G, :].rearrange(
                            "(tb tl) p -> tb tl p", tl=R_IN
                        )
                        eng = nc.scalar if h % 2 == 0 else nc.sync
                        eng.dma_start(out=dst_h, in_=ostg_h[:])
                    return
                ps_out = poutp.tile([P, SC], dt.float32, name="ps_out")
                for tl in range(R_OUT):
                    # strided stationary: times with residue tl (mod 8)
                    nc.tensor.transpose(
                        out=ps_out[:, tl * BLK:(tl + 1) * BLK],
                        in_=s_t[:, tl::R_OUT],
                        identity=ident[:],
                    )
                ostg = outp.tile([P, SC], dt.float32, name="ostg")
                nc.scalar.activation(
                    out=ostg[:], in_=ps_out[:],
                    func=mybir.ActivationFunctionType.Copy,
                )
                # t = tb*8 + tl; partition = tb; free = [tl, p]
                dst = chunk_out.rearrange("(tb tl) p -> tb tl p", tl=R_OUT)
                # mid-run: out-DMAs alternate the Scalar HWDGE and GpSimd
                # SWDGE rings, keeping the Sync ring for in-DMAs only — each
                # FIFO ring then carries <= 16.8 MB and none becomes the
                # serial tail
                eng = nc.scalar if k % 2 == 0 else nc.gpsimd
                eng.dma_start(out=dst, in_=ostg[:])

            for k in range(len(iters)):
                emit_in(k)
                if k >= PIPE:
                    emit_out(k - PIPE)
            for k in range(len(iters) - PIPE, len(iters)):
                emit_out(k)

    nc.compile()
    return nc


def get_nc(b_per=B_PER, seq_len=L):
    key = (b_per, seq_len)
    if key not in _nc_cache:
        _nc_cache[key] = _build_nc(b_per, seq_len)
    return _nc_cache[key]


def kernel(x: np.ndarray, w: np.ndarray, trace: bool = False):
    from concourse.bass_utils import run_bass_kernel_spmd

    x = np.ascontiguousarray(np.asarray(x), dtype=np.float32)
    w = np.ascontiguousarray(np.asarray(w), dtype=np.float32).reshape(P, 1)
    assert x.shape == (B, L, P), x.shape

    nc = get_nc()
    in_maps = [
        {"x": np.ascontiguousarray(x[i * B_PER:(i + 1) * B_PER]), "w": w}
        for i in range(N_CORES)
    ]
    res = run_bass_kernel_spmd(nc, in_maps, core_ids=list(range(N_CORES)), trace=trace)
    out = np.concatenate([r["out"] for r in res.results], axis=0)
    if trace:
        return out, res
    return out


# revision 41
# speedup vs baseline: 1.0146x; 1.0146x over previous
"""Diagonal RNN associative scan on 8 TRN2 NeuronCores.

Math (per batch row b, channel p):
    a[p]   = 1 - relu(w[p])
    h[t]   = a[p] * h[t-1] + x[b, t, p],   h[-1] = 0
    out[b, t, p] = h[t]

Strategy:
  - Data-parallel over batch: B=32 rows -> 8 cores x 4 rows. No collectives.
  - Per core, per (batch row, 1024-step scan chunk): DMA in, PE-transpose so
    channels sit on partitions, run the hardware linear-recurrence
    instruction (tensor_tensor_scan) along the free (time) axis in fp32
    carrying h across chunks, PE-transpose back, ACT-copy PSUM->SBUF, DMA out.
  - DMA descriptor-friendly layouts: time rows interleaved per partition so
    descriptors move 2 KiB (in, R=4) / 4 KiB (out, R=8) contiguous instead
    of 512B. The interleave is undone on-chip: in-path transposes write PSUM
    with stride-4 column APs (time-major); out-path transposes read the scan
    output with stride-8 free APs (strided stationary), writing PSUM
    contiguously.
  - Software-pipelined emission: the out-path of chunk k-2 is emitted after
    the in-path of chunk k, so the PE never head-of-line blocks on the scan
    of its own chunk (PE executes its FIFO in order; interleaving keeps it
    busy and lets the clock ramp).
  - in-DMAs issue on the Sync HWDGE ring, out-DMAs alternate between the
    Scalar and Sync rings (a single ring serializes transfers).
"""

import numpy as np

B, L, P = 32, 8192, 128
N_CORES = 8
B_PER = B // N_CORES  # 4 batch rows per core
SC = 1024             # scan-chunk time steps (one DMA + one scan instruction)
G = 512               # in-path interleave group (one PSUM bank of f32)
BLK = 128             # PE transpose tile edge
R_IN = 4              # in-path time-row interleave (2 KiB descriptors).
                      # R_IN=8 f32 needs bank-crossing strided PSUM matmul
                      # writes (NRT_EXEC_UNIT_UNRECOVERABLE on HW); bf16
                      # in-path hits the 4-byte PSUM alignment rule for odd
                      # residues. 2 KiB is the legal in-path maximum.
R_OUT = 8             # out-path time-row interleave (4 KiB descriptors)
PIPE = 2              # out-path emission delay in iterations

_nc_cache = {}


def _build_nc(b_per=B_PER, seq_len=L):
    """Build + compile the per-core Bass program (SPMD; same NEFF on all cores)."""
    import concourse.mybir as mybir
    import concourse.tile as tile
    from concourse import bacc
    from concourse.masks import make_identity

    dt = mybir.dt
    n_ch = seq_len // SC       # scan chunks per batch row
    n_grp = SC // G            # in-path 512-groups per scan chunk
    assert seq_len % SC == 0 and SC % G == 0 and G // BLK == R_IN

    nc = bacc.Bacc("TRN2", target_bir_lowering=False, debug=False)
    x_ext = nc.dram_tensor("x", [b_per, seq_len, P], dt.float32, kind="ExternalInput")
    w_ext = nc.dram_tensor("w", [P, 1], dt.float32, kind="ExternalInput")
    y_ext = nc.dram_tensor("out", [b_per, seq_len, P], dt.float32, kind="ExternalOutput")

    with tile.TileContext(nc) as tc:
        with (
            tc.tile_pool(name="const", bufs=1) as constp,
            tc.tile_pool(name="xin", bufs=8) as inp,
            tc.tile_pool(name="scan", bufs=12) as scanp,
            tc.tile_pool(name="ostg", bufs=6) as outp,
            tc.tile_pool(name="ps_in", bufs=2, space="PSUM") as pinp,
            tc.tile_pool(name="ps_out", bufs=2, space="PSUM") as poutp,
        ):
            ident = constp.tile([P, P], dt.float32, name="ident")
            make_identity(nc, ident[:])

            # w DMA on the gpsimd (SWDGE) ring so the Sync ring's first
            # instruction is iteration 0's x-chunk DMA
            w_col = constp.tile([P, 1], dt.float32, name="w_col")
            nc.gpsimd.dma_start(out=w_col[:], in_=w_ext.ap())
            a_col = constp.tile([P, 1], dt.float32, name="a_col")
            # a = 1 - relu(w)  ==  (max(w, 0) * -1) + 1
            nc.vector.tensor_scalar(
                out=a_col[:], in0=w_col[:], scalar1=0.0, scalar2=None,
                op0=mybir.AluOpType.max,
            )
            nc.vector.tensor_scalar(
                out=a_col[:], in0=a_col[:], scalar1=-1.0, scalar2=1.0,
                op0=mybir.AluOpType.mult, op1=mybir.AluOpType.add,
            )
            # scan's data0 operand: a replicated along the time axis
            a_rep = constp.tile([P, SC], dt.float32, name="a_rep")
            nc.vector.tensor_copy(out=a_rep[:], in_=a_col[:].to_broadcast([P, SC]))

            x_ap = x_ext.ap()
            y_ap = y_ext.ap()
            carry = [None] * b_per
            iters = [(c, b) for c in range(n_ch) for b in range(b_per)]
            state = {}  # k -> s_t tile

            def emit_in(k):
                c, b = iters[k]
                chunk_in = x_ap[b, c * SC:(c + 1) * SC, :]
                xin = inp.tile([P, SC], dt.float32, name="xin")
                if k < 2:
                    # pipeline warm-up: land the chunk in per-group pieces so
                    # the first transposes start after 256 KiB, not 512 KiB
                    for g in range(n_grp):
                        src_g = chunk_in[g * G:(g + 1) * G, :].rearrange(
                            "(tb tl) p -> tb tl p", tl=R_IN
                        )
                        nc.sync.dma_start(out=xin[:, g * G:(g + 1) * G], in_=src_g)
                else:
                    # free = [grp, tl, p]; partition = tb; t = grp*512 + tb*4 + tl
                    src = chunk_in.rearrange(
                        "(grp tb tl) p -> tb grp tl p", grp=n_grp, tl=R_IN
                    )
                    nc.sync.dma_start(out=xin[:], in_=src)

                ps_in = pinp.tile([P, SC], dt.float32, name="ps_in")
                for g in range(n_grp):
                    for tl in range(R_IN):
                        # stride-4 psum columns: time-major within group g
                        nc.tensor.transpose(
                            out=ps_in[:, g * G + tl:(g + 1) * G:R_IN],
                            in_=xin[:, (g * R_IN + tl) * BLK:(g * R_IN + tl + 1) * BLK],
                            identity=ident[:],
                        )

                s_t = scanp.tile([P, SC], dt.float32, name="s_t")
                init = 0.0 if carry[b] is None else carry[b]
                nc.vector.tensor_tensor_scan(
                    out=s_t[:], data0=a_rep[:], data1=ps_in[:],
                    initial=init,
                    op0=mybir.AluOpType.mult, op1=mybir.AluOpType.add,
                )
                carry[b] = s_t[:, SC - 1:SC]
                state[k] = s_t

            def emit_out(k):
                c, b = iters[k]
                s_t = state.pop(k)
                chunk_out = y_ap[b, c * SC:(c + 1) * SC, :]
                if k >= len(iters) - 2:
                    # pipeline drain: process the chunk in two pipelined
                    # halves (R=4, 2 KiB descriptors) so the final DMA isn't
                    # gated on the whole chunk's transposes + copy
                    for h in range(2):
                        ps_h = poutp.tile([P, G], dt.float32, name="ps_out")
                        for tl in range(R_IN):
                            nc.tensor.transpose(
                                out=ps_h[:, tl * BLK:(tl + 1) * BLK],
                                in_=s_t[:, h * G + tl:(h + 1) * G:R_IN],
                                identity=ident[:],
                            )
                        ostg_h = outp.tile([P, G], dt.float32, name="ostg")
                        nc.scalar.activation(
                            out=ostg_h[:], in_=ps_h[:],
                            func=mybir.ActivationFunctionType.Copy,
                        )
                        dst_h = chunk_out[h * G:(h + 1) * G, :].rearrange(
                            "(tb tl) p -> tb tl p", tl=R_IN
                        )
                        eng = nc.scalar if h % 2 == 0 else nc.sync
                        eng.dma_start(out=dst_h, in_=ostg_h[:])
                    return
                ps_out = poutp.tile([P, SC], dt.float32, name="ps_out")
                for tl in range(R_OUT):
                    # strided stationary: times with residue tl (mod 8)
                    nc.tensor.transpose(
                        out=ps_out[:, tl * BLK:(tl + 1) * BLK],
                        in_=s_t[:, tl::R_OUT],
                        identity=ident[:],
                    )
                ostg = outp.tile([P, SC], dt.float32, name="ostg")
                nc.scalar.activation(
                    out=ostg[:], in_=ps_out[:],
                    func=mybir.ActivationFunctionType.Copy,
                )
                # t = tb*8 + tl; partition = tb; free = [tl, p]
                dst = chunk_out.rearrange("(tb tl) p -> tb tl p", tl=R_OUT)
                # mid-run: out-DMAs alternate the Scalar HWDGE and GpSimd
                # SWDGE rings, keeping the Sync ring for in-DMAs only — each
                # FIFO ring then carries <= 16.8 MB and none becomes the
                # serial tail
                eng = nc.scalar if k % 2 == 0 else nc.gpsimd
                eng.dma_start(out=dst, in_=ostg[:])

            for k in range(len(iters)):
                emit_in(k)
                if k >= PIPE:
                    emit_out(k - PIPE)
            for k in range(len(iters) - PIPE, len(iters)):
                emit_out(k)

    nc.compile()
    return nc


def get_nc(b_per=B_PER, seq_len=L):
    key = (b_per, seq_len)
    if key not in _nc_cache:
        _nc_cache[key] = _build_nc(b_per, seq_len)
    return _nc_cache[key]


def kernel(x: np.ndarray, w: np.ndarray, trace: bool = False):
    from concourse.bass_utils import run_bass_kernel_spmd

    x = np.ascontiguousarray(np.asarray(x), dtype=np.float32)
    w = np.ascontiguousarray(np.asarray(w), dtype=np.float32).reshape(P, 1)
    assert x.shape == (B, L, P), x.shape

    nc = get_nc()
    in_maps = [
        {"x": np.ascontiguousarray(x[i * B_PER:(i + 1) * B_PER]), "w": w}
        for i in range(N_CORES)
    ]
    res = run_bass_kernel_spmd(nc, in_maps, core_ids=list(range(N_CORES)), trace=trace)
    out = np.concatenate([r["out"] for r in res.results], axis=0)
    if trace:
        return out, res
    return out
